# revision 12
# baseline (speedup 1.0000x reference)
"""Trainium2 Bass kernel for nn_DirectionalAttention (B=8,S=1024,D=1024,H=16).

Sharding: pure data-parallel over batch — 8 batch elements onto 8 NeuronCores,
zero collectives. Each core runs the full attention layer for one batch
element. Host pre-transposes activations/weights so no on-device input
transposes are needed; the out-proj bias is folded as bo' = bv@wo.T + bo
(valid because softmax rows sum to 1) into the residual input.

Per-core dataflow (S=1024, D=1024, H=16, DK=64), f32r matmuls (full PE rate,
~1.5e-4 rel err measured on HW; flip MM_DT to float32 for 1e-7 at 4x cost):
  QT = wq @ x^T [D,S], KT likewise, V = v @ wv^T [S,D]
  per (head, q-tile): scores -> +mask (mixed blocks) -> ACT Exp(scale=1/8,
    accum_out=rowsum) -> reciprocal -> normalize (DVE 2x) -> DMA out
    (skipped all-masked blocks stay zero: output buffers are pre-zeroed)
    -> PE-transpose 128-blocks -> attnT (f32r)
  AV computed transposed: AVT[d,q] = Vh.T @ attnT — directly the lhsT layout
    the output projection needs; odd heads land on PSUM partitions 64:128
    via tile_position col groups.
  y = LN(AVT.T @ woT + query + bo') * gamma + beta  (bn_stats/bn_aggr)

Self-contained: hardcodes shapes; reads nothing from /root/problem.
"""

import numpy as np

import concourse.bacc as bacc
import concourse.bass as bass
import concourse.mybir as mybir
from concourse.tile import TileContext
from concourse.bass_utils import run_bass_kernel_spmd

B, S, D, H = 8, 1024, 1024, 16
DK = D // H
P = 128
NT = S // P
LN_EPS = 1e-5
NEG = -1e9

MM_DT = mybir.dt.float32r
F32 = mybir.dt.float32
FP16 = mybir.dt.float16
AFT = mybir.ActivationFunctionType

_CACHE: dict = {}


def _classify_blocks(mask: np.ndarray):
    kinds, mixed = {}, {}
    for qi in range(NT):
        for ki in range(NT):
            blk = mask[qi * P:(qi + 1) * P, ki * P:(ki + 1) * P]
            if (blk != 0).all():
                kinds[qi, ki] = "one"
            elif (blk == 0).all():
                kinds[qi, ki] = "zero"
            else:
                kinds[qi, ki] = "mix"
                mixed[qi, ki] = np.where(blk == 0, np.float32(NEG),
                                         np.float32(0.0))
    return kinds, mixed


def _build(kinds, n_mixed):
    nc = bacc.Bacc("TRN2", target_bir_lowering=False, debug=False,
                   num_devices=B)
    dp = nc.declare_dram_parameter
    xT = dp("xT", [D, S], F32, isOutput=False)
    kTd = dp("kT", [D, S], F32, isOutput=False)
    vT = dp("vT", [D, S], F32, isOutput=False)
    xq = dp("xq", [S, D], F32, isOutput=False)
    wqT = dp("wqT", [D, D], F32, isOutput=False)
    wkT = dp("wkT", [D, D], F32, isOutput=False)
    wvT = dp("wvT", [D, D], F32, isOutput=False)
    woT = dp("woT", [D, D], F32, isOutput=False)
    bq8 = dp("bq8", [P, NT], F32, isOutput=False)
    bk8 = dp("bk8", [P, NT], F32, isOutput=False)
    gam = dp("gam", [1, D], F32, isOutput=False)
    bet = dp("bet", [1, D], F32, isOutput=False)
    mixm = dp("mixm", [max(n_mixed, 1), P, P], F32, isOutput=False)
    y_d = dp("y", [S, D], F32, isOutput=True)
    attn_d = dp("attn", [H, S, S], F32, isOutput=True)

    kblocks = {qi: [ki for ki in range(NT) if kinds[qi, ki] != "zero"]
               for qi in range(NT)}
    for qi in range(NT):
        assert kblocks[qi], "fully-masked row tile: softmax undefined"
    mix_idx = {}
    for qi in range(NT):
        for ki in range(NT):
            if kinds[qi, ki] == "mix":
                mix_idx[qi, ki] = len(mix_idx)

    with TileContext(nc) as tc:
      with tc.tile_pool(name="consts", bufs=1) as cp, \
           tc.tile_pool(name="stat", bufs=6) as st:
        gam_b = cp.tile([P, D], F32)
        bet_b = cp.tile([P, D], F32)
        with tc.tile_pool(name="rowstage", bufs=1) as rsp:
            grow = rsp.tile([1, D], F32, tag="grow")
            brow = rsp.tile([1, D], F32, tag="brow")
            nc.sync.dma_start(grow, gam[:, :])
            nc.sync.dma_start(brow, bet[:, :])
            nc.gpsimd.partition_broadcast(gam_b, grow)
            nc.gpsimd.partition_broadcast(bet_b, brow)
        bq_sb = cp.tile([P, NT], F32)
        bk_sb = cp.tile([P, NT], F32)
        nc.sync.dma_start(bq_sb, bq8[:, :])
        nc.sync.dma_start(bk_sb, bk8[:, :])
        eps_sb = cp.tile([P, 1], F32)
        nc.vector.memset(eps_sb, LN_EPS)
        masks_sb = cp.tile([P, max(n_mixed, 1), P], F32)
        nc.sync.dma_start(masks_sb, mixm[:, :, :].rearrange("n p m -> p n m"))

        with tc.tile_pool(name="avt", bufs=1) as ap:
          AVT = ap.tile([P, NT, S], FP16)

          with tc.tile_pool(name="qkv", bufs=1) as qp:
            QT = qp.tile([P, NT, S], MM_DT)
            KT = qp.tile([P, NT, S], MM_DT)
            V = qp.tile([P, NT, D], FP16)

            # ============ projections (k-outer, 8 live psum banks) ========
            with tc.tile_pool(name="wres", bufs=1) as wr, \
                 tc.tile_pool(name="praw", bufs=3) as raw, \
                 tc.tile_pool(name="pcast", bufs=3) as cst, \
                 tc.tile_pool(name="ppsum", bufs=8, space="PSUM") as pps:

                def project(w_dram, x_dram, out_sb, bias_sb, w_is_lhsT):
                    w_r = wr.tile([P, NT, D], MM_DT, tag="wres")
                    for k in range(NT):
                        wt = raw.tile([P, D], F32, tag="rawW")
                        nc.sync.dma_start(wt, w_dram[bass.ds(k * P, P), :])
                        nc.vector.tensor_copy(w_r[:, k], wt)
                    for mg in range(2):
                        pss = [pps.tile([P, 512], F32, tag="pj",
                                        name=f"pj{i}")
                               for i in range(8)]
                        for k in range(NT):
                            xt = raw.tile([P, S], F32, tag="rawX")
                            nc.sync.dma_start(
                                xt, x_dram[bass.ds(k * P, P), :])
                            xc = cst.tile([P, S], MM_DT, tag="castX")
                            nc.vector.tensor_copy(xc, xt)
                            for mi in range(4):
                                m = mg * 4 + mi
                                for n in range(2):
                                    if w_is_lhsT:
                                        lhsT = w_r[:, k, m * P:(m + 1) * P]
                                        rhs = xc[:, n * 512:(n + 1) * 512]
                                    else:
                                        lhsT = xc[:, m * P:(m + 1) * P]
                                        rhs = w_r[:, k, n * 512:(n + 1) * 512]
                                    nc.tensor.matmul(
                                        pss[mi * 2 + n], lhsT, rhs,
                                        start=(k == 0), stop=(k == NT - 1))
                        for mi in range(4):
                            m = mg * 4 + mi
                            for n in range(2):
                                dst = out_sb[:, m, n * 512:(n + 1) * 512]
                                ps = pss[mi * 2 + n]
                                if bias_sb is not None:
                                    nc.vector.tensor_scalar_add(
                                        dst, ps, bias_sb[:, m:m + 1])
                                else:
                                    nc.vector.tensor_copy(dst, ps)

                project(wqT, xT, QT, bq_sb, True)
                project(wkT, kTd, KT, bk_sb, True)
                project(wvT, vT, V, None, False)

            # ===================== attention ==============================
            with tc.tile_pool(name="atile", bufs=1) as tp, \
                 tc.tile_pool(name="awork", bufs=3) as wkp, \
                 tc.tile_pool(name="psc", bufs=3, space="PSUM") as psc, \
                 tc.tile_pool(name="pav", bufs=2, space="PSUM") as pavp:
                attnT = tp.tile([P, NT, S], FP16)
                zero_f = tp.tile([P, S], F32, tag="zf")
                nc.vector.memset(zero_f, 0.0)
                for ki in range(NT):
                    if any(kinds[qi, ki] == "zero" for qi in range(NT)):
                        nc.vector.tensor_copy(attnT[:, ki], zero_f)

                for h in range(H):
                    ht, hp = divmod(h * DK, P)
                    for c in range(2):
                        for qi in range(4 * c, 4 * c + 4):
                            kbs = kblocks[qi]
                            wmax = (max(kbs) + 1) * P
                            ps = psc.tile([P, S], F32, tag="psc")
                            # contiguous runs of needed k-blocks, split at
                            # 512-aligned PSUM bank boundaries
                            for g in range(2):
                                grp = [ki for ki in kbs
                                       if g * 4 <= ki < g * 4 + 4]
                                while grp:
                                    a = grp[0]
                                    b2 = a
                                    while b2 + 1 in grp:
                                        b2 += 1
                                    grp = [x for x in grp if x > b2]
                                    nc.tensor.matmul(
                                        ps[:, a * P:(b2 + 1) * P],
                                        QT[hp:hp + DK, ht,
                                           qi * P:(qi + 1) * P],
                                        KT[hp:hp + DK, ht,
                                           a * P:(b2 + 1) * P],
                                        start=True, stop=True)
                            for ki in range(wmax // P):
                                if ki not in kbs:
                                    nc.vector.memset(
                                        ps[:, ki * P:(ki + 1) * P], NEG)
                                elif kinds[qi, ki] == "mix":
                                    nc.vector.tensor_add(
                                        ps[:, ki * P:(ki + 1) * P],
                                        ps[:, ki * P:(ki + 1) * P],
                                        masks_sb[:, mix_idx[qi, ki]])
                            ex = wkp.tile([P, S], F32, tag="exp")
                            rs = st.tile([P, 1], F32, tag="rs")
                            rv = st.tile([P, 1], F32, tag="rv")
                            at = wkp.tile([P, S], F32, tag="attn")
                            nc.scalar.activation(
                                ex[:, :wmax], ps[:, :wmax], AFT.Exp,
                                scale=0.125, accum_out=rs)
                            nc.vector.reciprocal(rv, rs)
                            nc.vector.tensor_scalar_mul(
                                at[:, :wmax], ex[:, :wmax], rv)
                            nc.sync.dma_start(
                                attn_d[h, bass.ds(qi * P, P), 0:wmax],
                                at[:, :wmax])
                            at16 = wkp.tile([P, S], FP16, tag="a16")
                            nc.scalar.mul(at16[:, :wmax], ex[:, :wmax],
                                          mul=rv)
                            for ki in kbs:
                                nc.sync.dma_start_transpose(
                                    attnT[:, ki, qi * P:(qi + 1) * P],
                                    at16[:, ki * P:(ki + 1) * P])
                        kis = sorted({k for qi in range(4 * c, 4 * c + 4)
                                      for k in kblocks[qi]})
                        # matmul dst must start at PSUM partition 0
                        # (s3d3_mm_valid_dst_partition); odd heads bounce to
                        # AVT partitions 64:128 via SBUF->SBUF DMA.
                        pa = pavp.tile([DK, 512], F32, tag="pav")
                        for j, ki in enumerate(kis):
                            nc.tensor.matmul(
                                pa,
                                V[:, ki, h * DK:(h + 1) * DK],
                                attnT[:, ki, c * 512:(c + 1) * 512],
                                start=(j == 0), stop=(j == len(kis) - 1))
                        if hp == 0:
                            nc.vector.tensor_copy(
                                AVT[0:DK, ht, c * 512:(c + 1) * 512], pa)
                        else:
                            bounce = wkp.tile([DK, 512], FP16, tag="bnc")
                            nc.vector.tensor_copy(bounce, pa)
                            nc.sync.dma_start(
                                AVT[hp:hp + DK, ht, c * 512:(c + 1) * 512],
                                bounce)

          # ============== output projection + residual + LN ===============
          with tc.tile_pool(name="ores", bufs=1) as owr, \
               tc.tile_pool(name="oraw", bufs=3) as oraw, \
               tc.tile_pool(name="owork", bufs=2) as ow, \
               tc.tile_pool(name="opsum", bufs=4, space="PSUM") as ops:
            wo_r = owr.tile([P, NT, D], FP16)
            for k in range(NT):
                wt = oraw.tile([P, D], F32, tag="rawW")
                nc.sync.dma_start(wt, woT[bass.ds(k * P, P), :])
                nc.vector.tensor_copy(wo_r[:, k], wt)
            for m in range(NT):
                xr = oraw.tile([P, D], F32, tag="rawX")
                nc.sync.dma_start(xr, xq[bass.ds(m * P, P), :])
                y0 = ow.tile([P, D], F32, tag="y0")
                for n in range(2):
                    ps = ops.tile([P, 512], F32, tag="oj")
                    for k in range(NT):
                        nc.tensor.matmul(
                            ps, AVT[:, k, m * P:(m + 1) * P],
                            wo_r[:, k, n * 512:(n + 1) * 512],
                            start=(k == 0), stop=(k == NT - 1))
                    nc.vector.tensor_add(
                        y0[:, n * 512:(n + 1) * 512], ps,
                        xr[:, n * 512:(n + 1) * 512])
                stats = st.tile([P, 2, 6], F32, tag="bns")
                mv = st.tile([P, 2], F32, tag="mv")
                for sg in range(2):
                    nc.vector.bn_stats(stats[:, sg],
                                       y0[:, sg * 512:(sg + 1) * 512])
                nc.vector.bn_aggr(mv, stats)
                sd = st.tile([P, 1], F32, tag="sd")
                nc.scalar.activation(sd, mv[:, 1:2], AFT.Sqrt, bias=eps_sb)
                rsd = st.tile([P, 1], F32, tag="rsd")
                nc.vector.reciprocal(rsd, sd)
                y1 = ow.tile([P, D], F32, tag="y1")
                nc.vector.tensor_scalar(
                    y1, y0, scalar1=mv[:, 0:1], scalar2=rsd,
                    op0=mybir.AluOpType.subtract, op1=mybir.AluOpType.mult)
                nc.vector.tensor_mul(y1, y1, gam_b)
                nc.gpsimd.tensor_add(y1, y1, bet_b)
                nc.sync.dma_start(y_d[bass.ds(m * P, P), :], y1)

    nc.compile()
    return nc


def _prepare(query, key, value, causal_mask, wq, bq, wk, bk, wv, bv,
             wo, bo, gamma, beta):
    query = np.asarray(query, np.float32)
    key = np.asarray(key, np.float32)
    value = np.asarray(value, np.float32)
    mask = np.asarray(causal_mask)
    wq, bq = np.asarray(wq, np.float32), np.asarray(bq, np.float32)
    wk, bk = np.asarray(wk, np.float32), np.asarray(bk, np.float32)
    wv, bv = np.asarray(wv, np.float32), np.asarray(bv, np.float32)
    wo, bo = np.asarray(wo, np.float32), np.asarray(bo, np.float32)
    gamma, beta = np.asarray(gamma, np.float32), np.asarray(beta, np.float32)

    kinds, mixed = _classify_blocks(mask)
    key_sig = tuple(sorted(kinds.items()))
    if key_sig not in _CACHE:
        _CACHE[key_sig] = _build(kinds, max(len(mixed), 1))
    nc = _CACHE[key_sig]

    mix_list = [mixed[qi, ki] for qi in range(NT) for ki in range(NT)
                if kinds[qi, ki] == "mix"]
    mixm = (np.stack(mix_list) if mix_list
            else np.zeros((1, P, P), np.float32))
    bo_p = (bv @ wo.T + bo).astype(np.float32)
    shared = {
        "wqT": np.ascontiguousarray(wq.T), "wkT": np.ascontiguousarray(wk.T),
        "wvT": np.ascontiguousarray(wv.T), "woT": np.ascontiguousarray(wo.T),
        "bq8": np.ascontiguousarray(bq.reshape(NT, P).T),
        "bk8": np.ascontiguousarray(bk.reshape(NT, P).T),
        "gam": np.ascontiguousarray(gamma.reshape(1, D)),
        "bet": np.ascontiguousarray(beta.reshape(1, D)),
        "mixm": np.ascontiguousarray(mixm),
    }
    in_maps = [{
        "xT": np.ascontiguousarray(query[b].T),
        "kT": np.ascontiguousarray(key[b].T),
        "vT": np.ascontiguousarray(value[b].T),
        "xq": np.ascontiguousarray(query[b] + bo_p),
        **shared,
    } for b in range(B)]
    return nc, in_maps


def kernel(**inputs):
    nc, in_maps = _prepare(**inputs)
    res = run_bass_kernel_spmd(nc, in_maps, list(range(B)))
    y = np.stack([res.results[b]["y"] for b in range(B)])
    attn = np.stack([res.results[b]["attn"] for b in range(B)])
    return y.astype(np.float32), attn.astype(np.float32)


def run_traced(inputs):
    """Like kernel() but with NTFF tracing; returns BassKernelResults."""
    nc, in_maps = _prepare(**inputs)
    return run_bass_kernel_spmd(nc, in_maps, list(range(B)), trace=True)


# revision 18
# speedup vs baseline: 2.2469x; 2.2469x over previous
"""Trainium2 Bass kernel for nn_DirectionalAttention (B=8,S=1024,D=1024,H=16).

Sharding: pure data-parallel over batch — 8 batch elements onto 8 NeuronCores,
zero collectives. Each core runs the full attention layer for one batch
element. Host pre-transposes activations/weights so no on-device input
transposes are needed; the out-proj bias is folded as bo' = bv@wo.T + bo
(valid because softmax rows sum to 1) into the residual input.

Per-core dataflow (S=1024, D=1024, H=16, DK=64), f32r matmuls (full PE rate,
~1.5e-4 rel err measured on HW; flip MM_DT to float32 for 1e-7 at 4x cost):
  QT = wq @ x^T [D,S], KT likewise, V = v @ wv^T [S,D]
  per (head, q-tile): scores -> +mask (mixed blocks) -> ACT Exp(scale=1/8,
    accum_out=rowsum) -> reciprocal -> normalize (DVE 2x) -> DMA out
    (skipped all-masked blocks stay zero: output buffers are pre-zeroed)
    -> PE-transpose 128-blocks -> attnT (f32r)
  AV computed transposed: AVT[d,q] = Vh.T @ attnT — directly the lhsT layout
    the output projection needs; odd heads land on PSUM partitions 64:128
    via tile_position col groups.
  y = LN(AVT.T @ woT + query + bo') * gamma + beta  (bn_stats/bn_aggr)

Self-contained: hardcodes shapes; reads nothing from /root/problem.
"""

import numpy as np

import concourse.bacc as bacc
import concourse.bass as bass
import concourse.mybir as mybir
from concourse.tile import TileContext
from concourse.masks import make_identity
from concourse.bass_utils import run_bass_kernel_spmd

B, S, D, H = 8, 1024, 1024, 16
DK = D // H
P = 128
NT = S // P
LN_EPS = 1e-5
NEG = -1e9

MM_DT = mybir.dt.float32r
F32 = mybir.dt.float32
FP16 = mybir.dt.float16
AFT = mybir.ActivationFunctionType

_CACHE: dict = {}


def _classify_blocks(mask: np.ndarray):
    kinds, mixed = {}, {}
    for qi in range(NT):
        for ki in range(NT):
            blk = mask[qi * P:(qi + 1) * P, ki * P:(ki + 1) * P]
            if (blk != 0).all():
                kinds[qi, ki] = "one"
            elif (blk == 0).all():
                kinds[qi, ki] = "zero"
            else:
                kinds[qi, ki] = "mix"
                mixed[qi, ki] = np.where(blk == 0, np.float32(NEG),
                                         np.float32(0.0))
    return kinds, mixed


def _build(kinds, n_mixed):
    nc = bacc.Bacc("TRN2", target_bir_lowering=False, debug=False,
                   num_devices=B)
    dp = nc.declare_dram_parameter
    xT = dp("xT", [D, S], F32, isOutput=False)
    kTd = dp("kT", [D, S], F32, isOutput=False)
    vT = dp("vT", [D, S], F32, isOutput=False)
    xq = dp("xq", [S, D], F32, isOutput=False)
    wqT = dp("wqT", [D, D], F32, isOutput=False)
    wkT = dp("wkT", [D, D], F32, isOutput=False)
    wvT = dp("wvT", [D, D], F32, isOutput=False)
    woT = dp("woT", [D, D], F32, isOutput=False)
    bq8 = dp("bq8", [P, NT], F32, isOutput=False)
    bk8 = dp("bk8", [P, NT], F32, isOutput=False)
    gam = dp("gam", [1, D], F32, isOutput=False)
    bet = dp("bet", [1, D], F32, isOutput=False)
    mixm = dp("mixm", [max(n_mixed, 1), P, P], F32, isOutput=False)
    mixmT = dp("mixmT", [max(n_mixed, 1), P, P], F32, isOutput=False)
    y_d = dp("y", [S, D], F32, isOutput=True)
    attn_d = dp("attn", [H, S, S], F32, isOutput=True)

    kblocks = {qi: [ki for ki in range(NT) if kinds[qi, ki] != "zero"]
               for qi in range(NT)}
    for qi in range(NT):
        assert kblocks[qi], "fully-masked row tile: softmax undefined"
    mix_idx = {}
    for qi in range(NT):
        for ki in range(NT):
            if kinds[qi, ki] == "mix":
                mix_idx[qi, ki] = len(mix_idx)

    with TileContext(nc) as tc:
      with tc.tile_pool(name="consts", bufs=1) as cp, \
           tc.tile_pool(name="stat", bufs=6) as st:
        ident = cp.tile([P, P], F32)
        make_identity(nc, ident)
        gam_b = cp.tile([P, D], F32)
        bet_b = cp.tile([P, D], F32)
        with tc.tile_pool(name="rowstage", bufs=1) as rsp:
            grow = rsp.tile([1, D], F32, tag="grow")
            brow = rsp.tile([1, D], F32, tag="brow")
            nc.sync.dma_start(grow, gam[:, :])
            nc.sync.dma_start(brow, bet[:, :])
            nc.gpsimd.partition_broadcast(gam_b, grow)
            nc.gpsimd.partition_broadcast(bet_b, brow)
        bq_sb = cp.tile([P, NT], F32)
        bk_sb = cp.tile([P, NT], F32)
        nc.sync.dma_start(bq_sb, bq8[:, :])
        nc.sync.dma_start(bk_sb, bk8[:, :])
        eps_sb = cp.tile([P, 1], F32)
        nc.vector.memset(eps_sb, LN_EPS)
        masks_sb = cp.tile([P, max(n_mixed, 1), P], F32)
        nc.sync.dma_start(masks_sb, mixm[:, :, :].rearrange("n p m -> p n m"))
        masksT_sb = cp.tile([P, max(n_mixed, 1), P], F32)
        nc.sync.dma_start(masksT_sb, mixmT[:, :, :].rearrange("n p m -> p n m"))

        with tc.tile_pool(name="avt", bufs=1) as ap:
          AVT = ap.tile([P, NT, S], FP16)

          with tc.tile_pool(name="qkv", bufs=1) as qp:
            QT = qp.tile([P, NT, S], MM_DT)
            KT = qp.tile([P, NT, S], MM_DT)
            V = qp.tile([P, NT, D], FP16)

            # ============ projections (k-outer, 8 live psum banks) ========
            with tc.tile_pool(name="wres", bufs=1) as wr, \
                 tc.tile_pool(name="praw", bufs=3) as raw, \
                 tc.tile_pool(name="pcast", bufs=3) as cst, \
                 tc.tile_pool(name="ppsum", bufs=8, space="PSUM") as pps:

                def project(w_dram, x_dram, out_sb, bias_sb, w_is_lhsT):
                    w_r = wr.tile([P, NT, D], MM_DT, tag="wres")
                    for k in range(NT):
                        wt = raw.tile([P, D], F32, tag="rawW")
                        nc.sync.dma_start(wt, w_dram[bass.ds(k * P, P), :])
                        nc.vector.tensor_copy(w_r[:, k], wt)
                    for mg in range(2):
                        pss = [pps.tile([P, 512], F32, tag="pj",
                                        name=f"pj{i}")
                               for i in range(8)]
                        for k in range(NT):
                            xt = raw.tile([P, S], F32, tag="rawX")
                            nc.sync.dma_start(
                                xt, x_dram[bass.ds(k * P, P), :])
                            xc = cst.tile([P, S], MM_DT, tag="castX")
                            nc.vector.tensor_copy(xc, xt)
                            for mi in range(4):
                                m = mg * 4 + mi
                                for n in range(2):
                                    if w_is_lhsT:
                                        lhsT = w_r[:, k, m * P:(m + 1) * P]
                                        rhs = xc[:, n * 512:(n + 1) * 512]
                                    else:
                                        lhsT = xc[:, m * P:(m + 1) * P]
                                        rhs = w_r[:, k, n * 512:(n + 1) * 512]
                                    nc.tensor.matmul(
                                        pss[mi * 2 + n], lhsT, rhs,
                                        start=(k == 0), stop=(k == NT - 1))
                        for mi in range(4):
                            m = mg * 4 + mi
                            for n in range(2):
                                dst = out_sb[:, m, n * 512:(n + 1) * 512]
                                ps = pss[mi * 2 + n]
                                if bias_sb is not None:
                                    nc.vector.tensor_scalar_add(
                                        dst, ps, bias_sb[:, m:m + 1])
                                else:
                                    nc.vector.tensor_copy(dst, ps)

                project(wqT, xT, QT, bq_sb, True)
                project(wkT, kTd, KT, bk_sb, True)
                project(wvT, vT, V, None, False)

            # ===================== attention ==============================
            with tc.tile_pool(name="atile", bufs=1) as tp, \
                 tc.tile_pool(name="awork", bufs=3) as wkp, \
                 tc.tile_pool(name="psc", bufs=2, space="PSUM") as psc, \
                 tc.tile_pool(name="pav", bufs=2, space="PSUM") as pavp:
                attnT = tp.tile([P, NT, S], FP16)
                rinv_hq = tp.tile([P, H * NT], F32)
                rinvT = tp.tile([P, H * NT], F32)
                zero_f = tp.tile([P, S], F32, tag="zf")
                nc.vector.memset(zero_f, 0.0)
                for ki in range(NT):
                    if any(kinds[qi, ki] == "zero" for qi in range(NT)):
                        nc.vector.tensor_copy(attnT[:, ki], zero_f)

                # PASS 1: scores/softmax in [q,k] layout; collect
                # per-(head,qtile) rowsum reciprocals as columns.
                for h in range(H):
                    ht, hp = divmod(h * DK, P)
                    if True:
                        for qi in range(NT):
                            kbs = kblocks[qi]
                            wmax = (max(kbs) + 1) * P
                            ps = psc.tile([P, S], F32, tag="psc")
                            # contiguous runs of needed k-blocks, split at
                            # 512-aligned PSUM bank boundaries
                            for g in range(2):
                                grp = [ki for ki in kbs
                                       if g * 4 <= ki < g * 4 + 4]
                                while grp:
                                    a = grp[0]
                                    b2 = a
                                    while b2 + 1 in grp:
                                        b2 += 1
                                    grp = [x for x in grp if x > b2]
                                    nc.tensor.matmul(
                                        ps[:, a * P:(b2 + 1) * P],
                                        QT[hp:hp + DK, ht,
                                           qi * P:(qi + 1) * P],
                                        KT[hp:hp + DK, ht,
                                           a * P:(b2 + 1) * P],
                                        start=True, stop=True)
                            for ki in range(wmax // P):
                                if ki not in kbs:
                                    nc.vector.memset(
                                        ps[:, ki * P:(ki + 1) * P], NEG)
                                elif kinds[qi, ki] == "mix":
                                    nc.vector.tensor_add(
                                        ps[:, ki * P:(ki + 1) * P],
                                        ps[:, ki * P:(ki + 1) * P],
                                        masks_sb[:, mix_idx[qi, ki]])
                            ex = wkp.tile([P, S], F32, tag="exp")
                            rs = st.tile([P, 1], F32, tag="rs")
                            at = wkp.tile([P, S], F32, tag="attn")
                            nc.scalar.activation(
                                ex[:, :wmax], ps[:, :wmax], AFT.Exp,
                                scale=0.125, accum_out=rs)
                            hq = h * NT + qi
                            nc.vector.reciprocal(
                                rinv_hq[:, hq:hq + 1], rs)
                            nc.vector.tensor_scalar_mul(
                                at[:, :wmax], ex[:, :wmax],
                                rinv_hq[:, hq:hq + 1])
                            nc.sync.dma_start(
                                attn_d[h, bass.ds(qi * P, P), 0:wmax],
                                at[:, :wmax])
                # rinv columns [q, (h,qi)] -> rows [(h,qi), q]
                pstr = pavp.tile([P, P], F32, tag="psT")
                nc.tensor.transpose(pstr, rinv_hq, ident)
                nc.vector.tensor_copy(rinvT, pstr)
                # PASS 2: scoresT on PE (operand swap) -> exp -> attnT fp16
                # (unnormalized); AV output normalized at copyback with the
                # broadcast rinv row.
                for h in range(H):
                    ht, hp = divmod(h * DK, P)
                    for c in range(2):
                        # stage 4 rinv rows to partition 0, then
                        # broadcast (partition_broadcast needs base 0);
                        # all on the otherwise-idle GpSimd engine.
                        hq0 = h * NT + 4 * c
                        rstage = wkp.tile([1, 512], F32, tag="rstg")
                        nc.gpsimd.dma_start(rstage,
                                            rinvT[hq0:hq0 + 4, :])
                        rbc = wkp.tile([DK, 512], F32, tag="rbc")
                        nc.gpsimd.partition_broadcast(rbc, rstage)
                        c0 = c * 512
                        for ki in range(NT):
                            qs = [qi for qi in range(4 * c, 4 * c + 4)
                                  if kinds[qi, ki] != "zero"]
                            if not qs:
                                continue
                            qlo, qhi = qs[0] * P - c0, (qs[-1] + 1) * P - c0
                            psT = pavp.tile([P, 512], F32, tag="psT")
                            nc.tensor.matmul(
                                psT[:, qlo:qhi],
                                KT[hp:hp + DK, ht, ki * P:(ki + 1) * P],
                                QT[hp:hp + DK, ht, c0 + qlo:c0 + qhi],
                                start=True, stop=True)
                            for qi in qs:
                                a = qi * P - c0
                                if kinds[qi, ki] == "mix":
                                    nc.vector.tensor_add(
                                        psT[:, a:a + P], psT[:, a:a + P],
                                        masksT_sb[:, mix_idx[qi, ki]])
                            nc.scalar.activation(
                                attnT[:, ki, c0 + qlo:c0 + qhi],
                                psT[:, qlo:qhi], AFT.Exp, scale=0.125)
                            if qs != list(range(qs[0], qs[-1] + 1)):
                                raise NotImplementedError(
                                    "non-contiguous valid q-blocks")
                        kis = sorted({k for qi in range(4 * c, 4 * c + 4)
                                      for k in kblocks[qi]})
                        # matmul dst must start at PSUM partition 0
                        # (s3d3_mm_valid_dst_partition); odd heads bounce to
                        # AVT partitions 64:128 via SBUF->SBUF DMA.
                        pa = pavp.tile([DK, 512], F32, tag="pav")
                        for j, ki in enumerate(kis):
                            nc.tensor.matmul(
                                pa,
                                V[:, ki, h * DK:(h + 1) * DK],
                                attnT[:, ki, c * 512:(c + 1) * 512],
                                start=(j == 0), stop=(j == len(kis) - 1))
                        if hp == 0:
                            nc.vector.tensor_mul(
                                AVT[0:DK, ht, c * 512:(c + 1) * 512],
                                pa, rbc)
                        else:
                            bounce = wkp.tile([DK, 512], FP16, tag="bnc")
                            nc.vector.tensor_mul(bounce, pa, rbc)
                            nc.sync.dma_start(
                                AVT[hp:hp + DK, ht, c * 512:(c + 1) * 512],
                                bounce)

          # ============== output projection + residual + LN ===============
          with tc.tile_pool(name="ores", bufs=1) as owr, \
               tc.tile_pool(name="oraw", bufs=3) as oraw, \
               tc.tile_pool(name="owork", bufs=2) as ow, \
               tc.tile_pool(name="opsum", bufs=4, space="PSUM") as ops:
            wo_r = owr.tile([P, NT, D], FP16)
            for k in range(NT):
                wt = oraw.tile([P, D], F32, tag="rawW")
                nc.sync.dma_start(wt, woT[bass.ds(k * P, P), :])
                nc.vector.tensor_copy(wo_r[:, k], wt)
            for m in range(NT):
                xr = oraw.tile([P, D], F32, tag="rawX")
                nc.sync.dma_start(xr, xq[bass.ds(m * P, P), :])
                y0 = ow.tile([P, D], F32, tag="y0")
                for n in range(2):
                    ps = ops.tile([P, 512], F32, tag="oj")
                    for k in range(NT):
                        nc.tensor.matmul(
                            ps, AVT[:, k, m * P:(m + 1) * P],
                            wo_r[:, k, n * 512:(n + 1) * 512],
                            start=(k == 0), stop=(k == NT - 1))
                    nc.vector.tensor_add(
                        y0[:, n * 512:(n + 1) * 512], ps,
                        xr[:, n * 512:(n + 1) * 512])
                stats = st.tile([P, 2, 6], F32, tag="bns")
                mv = st.tile([P, 2], F32, tag="mv")
                for sg in range(2):
                    nc.vector.bn_stats(stats[:, sg],
                                       y0[:, sg * 512:(sg + 1) * 512])
                nc.vector.bn_aggr(mv, stats)
                sd = st.tile([P, 1], F32, tag="sd")
                nc.scalar.activation(sd, mv[:, 1:2], AFT.Sqrt, bias=eps_sb)
                rsd = st.tile([P, 1], F32, tag="rsd")
                nc.vector.reciprocal(rsd, sd)
                y1 = ow.tile([P, D], F32, tag="y1")
                nc.vector.tensor_scalar(
                    y1, y0, scalar1=mv[:, 0:1], scalar2=rsd,
                    op0=mybir.AluOpType.subtract, op1=mybir.AluOpType.mult)
                nc.vector.tensor_mul(y1, y1, gam_b)
                nc.gpsimd.tensor_add(y1, y1, bet_b)
                nc.sync.dma_start(y_d[bass.ds(m * P, P), :], y1)

    nc.compile()
    return nc


def _prepare(query, key, value, causal_mask, wq, bq, wk, bk, wv, bv,
             wo, bo, gamma, beta):
    query = np.asarray(query, np.float32)
    key = np.asarray(key, np.float32)
    value = np.asarray(value, np.float32)
    mask = np.asarray(causal_mask)
    wq, bq = np.asarray(wq, np.float32), np.asarray(bq, np.float32)
    wk, bk = np.asarray(wk, np.float32), np.asarray(bk, np.float32)
    wv, bv = np.asarray(wv, np.float32), np.asarray(bv, np.float32)
    wo, bo = np.asarray(wo, np.float32), np.asarray(bo, np.float32)
    gamma, beta = np.asarray(gamma, np.float32), np.asarray(beta, np.float32)

    kinds, mixed = _classify_blocks(mask)
    key_sig = tuple(sorted(kinds.items()))
    if key_sig not in _CACHE:
        _CACHE[key_sig] = _build(kinds, max(len(mixed), 1))
    nc = _CACHE[key_sig]

    mix_list = [mixed[qi, ki] for qi in range(NT) for ki in range(NT)
                if kinds[qi, ki] == "mix"]
    mixm = (np.stack(mix_list) if mix_list
            else np.zeros((1, P, P), np.float32))
    mixmT = np.ascontiguousarray(mixm.transpose(0, 2, 1))
    bo_p = (bv @ wo.T + bo).astype(np.float32)
    shared = {
        "wqT": np.ascontiguousarray(wq.T), "wkT": np.ascontiguousarray(wk.T),
        "wvT": np.ascontiguousarray(wv.T), "woT": np.ascontiguousarray(wo.T),
        "bq8": np.ascontiguousarray(bq.reshape(NT, P).T),
        "bk8": np.ascontiguousarray(bk.reshape(NT, P).T),
        "gam": np.ascontiguousarray(gamma.reshape(1, D)),
        "bet": np.ascontiguousarray(beta.reshape(1, D)),
        "mixm": np.ascontiguousarray(mixm),
        "mixmT": mixmT,
    }
    in_maps = [{
        "xT": np.ascontiguousarray(query[b].T),
        "kT": np.ascontiguousarray(key[b].T),
        "vT": np.ascontiguousarray(value[b].T),
        "xq": np.ascontiguousarray(query[b] + bo_p),
        **shared,
    } for b in range(B)]
    return nc, in_maps


def kernel(**inputs):
    nc, in_maps = _prepare(**inputs)
    res = run_bass_kernel_spmd(nc, in_maps, list(range(B)))
    y = np.stack([res.results[b]["y"] for b in range(B)])
    attn = np.stack([res.results[b]["attn"] for b in range(B)])
    return y.astype(np.float32), attn.astype(np.float32)


def run_traced(inputs):
    """Like kernel() but with NTFF tracing; returns BassKernelResults."""
    nc, in_maps = _prepare(**inputs)
    return run_bass_kernel_spmd(nc, in_maps, list(range(B)), trace=True)


# revision 20
# speedup vs baseline: 2.5016x; 1.1134x over previous
"""Trainium2 Bass kernel for nn_DirectionalAttention (B=8,S=1024,D=1024,H=16).

Sharding: pure data-parallel over batch — 8 batch elements onto 8 NeuronCores,
zero collectives. Each core runs the full attention layer for one batch
element. Host pre-transposes activations/weights so no on-device input
transposes are needed; the out-proj bias is folded as bo' = bv@wo.T + bo
(valid because softmax rows sum to 1) into the residual input.

Per-core dataflow (S=1024, D=1024, H=16, DK=64), f32r matmuls (full PE rate,
~1.5e-4 rel err measured on HW; flip MM_DT to float32 for 1e-7 at 4x cost):
  QT = wq @ x^T [D,S], KT likewise, V = v @ wv^T [S,D]
  per (head, q-tile): scores -> +mask (mixed blocks) -> ACT Exp(scale=1/8,
    accum_out=rowsum) -> reciprocal -> normalize (DVE 2x) -> DMA out
    (skipped all-masked blocks stay zero: output buffers are pre-zeroed)
    -> PE-transpose 128-blocks -> attnT (f32r)
  AV computed transposed: AVT[d,q] = Vh.T @ attnT — directly the lhsT layout
    the output projection needs; odd heads land on PSUM partitions 64:128
    via tile_position col groups.
  y = LN(AVT.T @ woT + query + bo') * gamma + beta  (bn_stats/bn_aggr)

Self-contained: hardcodes shapes; reads nothing from /root/problem.
"""

import numpy as np

import concourse.bacc as bacc
import concourse.bass as bass
import concourse.mybir as mybir
from concourse.tile import TileContext
from concourse.masks import make_identity
from concourse.bass_utils import run_bass_kernel_spmd

B, S, D, H = 8, 1024, 1024, 16
DK = D // H
P = 128
NT = S // P
LN_EPS = 1e-5
NEG = -1e9

MM_DT = mybir.dt.float32r
F32 = mybir.dt.float32
FP16 = mybir.dt.float16
AFT = mybir.ActivationFunctionType

_CACHE: dict = {}


def _classify_blocks(mask: np.ndarray):
    kinds, mixed = {}, {}
    for qi in range(NT):
        for ki in range(NT):
            blk = mask[qi * P:(qi + 1) * P, ki * P:(ki + 1) * P]
            if (blk != 0).all():
                kinds[qi, ki] = "one"
            elif (blk == 0).all():
                kinds[qi, ki] = "zero"
            else:
                kinds[qi, ki] = "mix"
                mixed[qi, ki] = np.where(blk == 0, np.float32(NEG),
                                         np.float32(0.0))
    return kinds, mixed


def _build(kinds, n_mixed):
    nc = bacc.Bacc("TRN2", target_bir_lowering=False, debug=False,
                   num_devices=B)
    dp = nc.declare_dram_parameter
    xT = dp("xT", [D, S], F32, isOutput=False)
    kTd = dp("kT", [D, S], F32, isOutput=False)
    vT = dp("vT", [D, S], F32, isOutput=False)
    xq = dp("xq", [S, D], F32, isOutput=False)
    wqT = dp("wqT", [D, D], F32, isOutput=False)
    wkT = dp("wkT", [D, D], F32, isOutput=False)
    wvT = dp("wvT", [D, D], F32, isOutput=False)
    woT = dp("woT", [D, D], F32, isOutput=False)
    bq8 = dp("bq8", [P, NT], F32, isOutput=False)
    bk8 = dp("bk8", [P, NT], F32, isOutput=False)
    gam = dp("gam", [1, D], F32, isOutput=False)
    bet = dp("bet", [1, D], F32, isOutput=False)
    mixm = dp("mixm", [max(n_mixed, 1), P, P], F32, isOutput=False)
    mixmT = dp("mixmT", [max(n_mixed, 1), P, P], F32, isOutput=False)
    y_d = dp("y", [S, D], F32, isOutput=True)
    attn_d = dp("attn", [H, S, S], F32, isOutput=True)

    kblocks = {qi: [ki for ki in range(NT) if kinds[qi, ki] != "zero"]
               for qi in range(NT)}
    for qi in range(NT):
        assert kblocks[qi], "fully-masked row tile: softmax undefined"
    mix_idx = {}
    for qi in range(NT):
        for ki in range(NT):
            if kinds[qi, ki] == "mix":
                mix_idx[qi, ki] = len(mix_idx)

    with TileContext(nc) as tc:
      with tc.tile_pool(name="consts", bufs=1) as cp, \
           tc.tile_pool(name="stat", bufs=6) as st:
        ident = cp.tile([P, P], F32)
        make_identity(nc, ident)
        gam_b = cp.tile([P, D], F32)
        bet_b = cp.tile([P, D], F32)
        with tc.tile_pool(name="rowstage", bufs=1) as rsp:
            grow = rsp.tile([1, D], F32, tag="grow")
            brow = rsp.tile([1, D], F32, tag="brow")
            nc.sync.dma_start(grow, gam[:, :])
            nc.sync.dma_start(brow, bet[:, :])
            nc.gpsimd.partition_broadcast(gam_b, grow)
            nc.gpsimd.partition_broadcast(bet_b, brow)
        bq_sb = cp.tile([P, NT], F32)
        bk_sb = cp.tile([P, NT], F32)
        nc.sync.dma_start(bq_sb, bq8[:, :])
        nc.sync.dma_start(bk_sb, bk8[:, :])
        eps_sb = cp.tile([P, 1], F32)
        nc.vector.memset(eps_sb, LN_EPS)
        masks_sb = cp.tile([P, max(n_mixed, 1), P], F32)
        nc.sync.dma_start(masks_sb, mixm[:, :, :].rearrange("n p m -> p n m"))
        masksT_sb = cp.tile([P, max(n_mixed, 1), P], F32)
        nc.sync.dma_start(masksT_sb, mixmT[:, :, :].rearrange("n p m -> p n m"))

        with tc.tile_pool(name="avt", bufs=1) as ap:
          AVT = ap.tile([P, NT, S], FP16)

          with tc.tile_pool(name="qkv", bufs=1) as qp:
            QT = qp.tile([P, NT, S], MM_DT)
            KT = qp.tile([P, NT, S], MM_DT)
            V = qp.tile([P, NT, D], FP16)

            # ============ projections (k-outer, 8 live psum banks) ========
            with tc.tile_pool(name="wres", bufs=1) as wr, \
                 tc.tile_pool(name="praw", bufs=3) as raw, \
                 tc.tile_pool(name="pcast", bufs=3) as cst, \
                 tc.tile_pool(name="ppsum", bufs=8, space="PSUM") as pps:

                def project(w_dram, x_dram, out_sb, bias_sb, w_is_lhsT):
                    w_r = wr.tile([P, NT, D], MM_DT, tag="wres")
                    for k in range(NT):
                        wt = raw.tile([P, D], F32, tag="rawW")
                        nc.sync.dma_start(wt, w_dram[bass.ds(k * P, P), :])
                        nc.vector.tensor_copy(w_r[:, k], wt)
                    for mg in range(2):
                        pss = [pps.tile([P, 512], F32, tag="pj",
                                        name=f"pj{i}")
                               for i in range(8)]
                        for k in range(NT):
                            xt = raw.tile([P, S], F32, tag="rawX")
                            nc.sync.dma_start(
                                xt, x_dram[bass.ds(k * P, P), :])
                            xc = cst.tile([P, S], MM_DT, tag="castX")
                            nc.vector.tensor_copy(xc, xt)
                            for mi in range(4):
                                m = mg * 4 + mi
                                for n in range(2):
                                    if w_is_lhsT:
                                        lhsT = w_r[:, k, m * P:(m + 1) * P]
                                        rhs = xc[:, n * 512:(n + 1) * 512]
                                    else:
                                        lhsT = xc[:, m * P:(m + 1) * P]
                                        rhs = w_r[:, k, n * 512:(n + 1) * 512]
                                    nc.tensor.matmul(
                                        pss[mi * 2 + n], lhsT, rhs,
                                        start=(k == 0), stop=(k == NT - 1))
                        for mi in range(4):
                            m = mg * 4 + mi
                            for n in range(2):
                                dst = out_sb[:, m, n * 512:(n + 1) * 512]
                                ps = pss[mi * 2 + n]
                                if bias_sb is not None:
                                    nc.vector.tensor_scalar_add(
                                        dst, ps, bias_sb[:, m:m + 1])
                                else:
                                    nc.vector.tensor_copy(dst, ps)

                project(wqT, xT, QT, bq_sb, True)
                project(wkT, kTd, KT, bk_sb, True)
                project(wvT, vT, V, None, False)

            # ===================== attention ==============================
            with tc.tile_pool(name="atile", bufs=1) as tp, \
                 tc.tile_pool(name="awork", bufs=3) as wkp, \
                 tc.tile_pool(name="psc", bufs=2, space="PSUM") as psc, \
                 tc.tile_pool(name="pstr", bufs=1, space="PSUM") as pstp, \
                 tc.tile_pool(name="pav", bufs=2, space="PSUM") as pavp:
                attnT = tp.tile([P, NT, S], FP16)
                rinv_hq = tp.tile([P, H * NT], F32)
                rinvT = tp.tile([P, H * NT], F32)
                zero_f = tp.tile([P, S], F32, tag="zf")
                nc.vector.memset(zero_f, 0.0)
                for ki in range(NT):
                    if any(kinds[qi, ki] == "zero" for qi in range(NT)):
                        nc.vector.tensor_copy(attnT[:, ki], zero_f)

                # Merged per-head loop with depth-1 skew: PE stays
                # dense (scores(h) -> scoresT(h) -> AV(h-1)) so ACT exp
                # latency hides and HAM stays warm. attnT double-buffered
                # so expT(h) doesn't clobber what AV(h-1) reads.
                attnT_bufs = [attnT, tp.tile([P, NT, S], FP16,
                                             name="attnT2")]
                for buf in attnT_bufs:
                    for ki in range(NT):
                        if any(kinds[qi, ki] == "zero"
                               for qi in range(NT)):
                            nc.vector.tensor_copy(buf[:, ki], zero_f)

                def emit_pass1(h):
                    ht, hp = divmod(h * DK, P)
                    for qi in range(NT):
                        kbs = kblocks[qi]
                        wmax = (max(kbs) + 1) * P
                        ps = psc.tile([P, S], F32, tag="psc", name="ps")
                        for g in range(2):
                            grp = [ki for ki in kbs
                                   if g * 4 <= ki < g * 4 + 4]
                            while grp:
                                a = grp[0]
                                b2 = a
                                while b2 + 1 in grp:
                                    b2 += 1
                                grp = [x for x in grp if x > b2]
                                nc.tensor.matmul(
                                    ps[:, a * P:(b2 + 1) * P],
                                    QT[hp:hp + DK, ht,
                                       qi * P:(qi + 1) * P],
                                    KT[hp:hp + DK, ht, a * P:(b2 + 1) * P],
                                    start=True, stop=True)
                        for ki in range(wmax // P):
                            if ki not in kbs:
                                nc.vector.memset(
                                    ps[:, ki * P:(ki + 1) * P], NEG)
                            elif kinds[qi, ki] == "mix":
                                nc.vector.tensor_add(
                                    ps[:, ki * P:(ki + 1) * P],
                                    ps[:, ki * P:(ki + 1) * P],
                                    masks_sb[:, mix_idx[qi, ki]])
                        ex = wkp.tile([P, S], F32, tag="exp", name="ex")
                        rs = st.tile([P, 1], F32, tag="rs", name="rs")
                        at = wkp.tile([P, S], F32, tag="attn", name="at")
                        nc.scalar.activation(
                            ex[:, :wmax], ps[:, :wmax], AFT.Exp,
                            scale=0.125, accum_out=rs)
                        hq = h * NT + qi
                        nc.vector.reciprocal(rinv_hq[:, hq:hq + 1], rs)
                        nc.vector.tensor_scalar_mul(
                            at[:, :wmax], ex[:, :wmax],
                            rinv_hq[:, hq:hq + 1])
                        nc.sync.dma_start(
                            attn_d[h, bass.ds(qi * P, P), 0:wmax],
                            at[:, :wmax])

                def emit_rbc(h):
                    pstr = pstp.tile([P, P], F32, tag="pstr", name="pstr")
                    nc.tensor.transpose(pstr, rinv_hq, ident)
                    nc.vector.tensor_copy(rinvT, pstr)
                    rbcs = []
                    for c in range(2):
                        hq0 = h * NT + 4 * c
                        rstage = wkp.tile([1, 512], F32, tag="rstg",
                                          name="rstage")
                        nc.gpsimd.dma_start(rstage,
                                            rinvT[hq0:hq0 + 4, :])
                        rbc = wkp.tile([DK, 512], F32, tag="rbc",
                                       name="rbc", bufs=4)
                        nc.gpsimd.partition_broadcast(rbc, rstage)
                        rbcs.append(rbc)
                    return rbcs

                def emit_scoresT(h, buf):
                    ht, hp = divmod(h * DK, P)
                    for c in range(2):
                        c0 = c * 512
                        for ki in range(NT):
                            qs = [qi for qi in range(4 * c, 4 * c + 4)
                                  if kinds[qi, ki] != "zero"]
                            if not qs:
                                continue
                            if qs != list(range(qs[0], qs[-1] + 1)):
                                raise NotImplementedError(
                                    "non-contiguous valid q-blocks")
                            qlo = qs[0] * P - c0
                            qhi = (qs[-1] + 1) * P - c0
                            psT = pavp.tile([P, 512], F32, tag="psT",
                                            name="psT")
                            nc.tensor.matmul(
                                psT[:, qlo:qhi],
                                KT[hp:hp + DK, ht, ki * P:(ki + 1) * P],
                                QT[hp:hp + DK, ht, c0 + qlo:c0 + qhi],
                                start=True, stop=True)
                            for qi in qs:
                                a = qi * P - c0
                                if kinds[qi, ki] == "mix":
                                    nc.vector.tensor_add(
                                        psT[:, a:a + P], psT[:, a:a + P],
                                        masksT_sb[:, mix_idx[qi, ki]])
                            nc.scalar.activation(
                                buf[:, ki, c0 + qlo:c0 + qhi],
                                psT[:, qlo:qhi], AFT.Exp, scale=0.125)

                def emit_av(h, buf, rbcs):
                    ht, hp = divmod(h * DK, P)
                    for c in range(2):
                        kis = sorted({k for qi in range(4 * c, 4 * c + 4)
                                      for k in kblocks[qi]})
                        pa = pavp.tile([DK, 512], F32, tag="pav",
                                       name="pa", bufs=1)
                        for j, ki in enumerate(kis):
                            nc.tensor.matmul(
                                pa, V[:, ki, h * DK:(h + 1) * DK],
                                buf[:, ki, c * 512:(c + 1) * 512],
                                start=(j == 0), stop=(j == len(kis) - 1))
                        if hp == 0:
                            nc.vector.tensor_mul(
                                AVT[0:DK, ht, c * 512:(c + 1) * 512],
                                pa, rbcs[c])
                        else:
                            bounce = wkp.tile([DK, 512], FP16, tag="bnc",
                                              name="bounce")
                            nc.vector.tensor_mul(bounce, pa, rbcs[c])
                            nc.sync.dma_start(
                                AVT[hp:hp + DK, ht,
                                    c * 512:(c + 1) * 512], bounce)

                prev = None
                for h in range(H):
                    buf = attnT_bufs[h % 2]
                    emit_pass1(h)
                    rbcs = emit_rbc(h)
                    emit_scoresT(h, buf)
                    if prev is not None:
                        emit_av(*prev)
                    prev = (h, buf, rbcs)
                assert prev is not None
                emit_av(*prev)

          # ============== output projection + residual + LN ===============
          with tc.tile_pool(name="ores", bufs=1) as owr, \
               tc.tile_pool(name="oraw", bufs=3) as oraw, \
               tc.tile_pool(name="owork", bufs=2) as ow, \
               tc.tile_pool(name="opsum", bufs=4, space="PSUM") as ops:
            wo_r = owr.tile([P, NT, D], FP16)
            for k in range(NT):
                wt = oraw.tile([P, D], F32, tag="rawW")
                nc.sync.dma_start(wt, woT[bass.ds(k * P, P), :])
                nc.vector.tensor_copy(wo_r[:, k], wt)
            for m in range(NT):
                xr = oraw.tile([P, D], F32, tag="rawX")
                nc.sync.dma_start(xr, xq[bass.ds(m * P, P), :])
                y0 = ow.tile([P, D], F32, tag="y0")
                for n in range(2):
                    ps = ops.tile([P, 512], F32, tag="oj")
                    for k in range(NT):
                        nc.tensor.matmul(
                            ps, AVT[:, k, m * P:(m + 1) * P],
                            wo_r[:, k, n * 512:(n + 1) * 512],
                            start=(k == 0), stop=(k == NT - 1))
                    nc.vector.tensor_add(
                        y0[:, n * 512:(n + 1) * 512], ps,
                        xr[:, n * 512:(n + 1) * 512])
                stats = st.tile([P, 2, 6], F32, tag="bns")
                mv = st.tile([P, 2], F32, tag="mv")
                for sg in range(2):
                    nc.vector.bn_stats(stats[:, sg],
                                       y0[:, sg * 512:(sg + 1) * 512])
                nc.vector.bn_aggr(mv, stats)
                sd = st.tile([P, 1], F32, tag="sd")
                nc.scalar.activation(sd, mv[:, 1:2], AFT.Sqrt, bias=eps_sb)
                rsd = st.tile([P, 1], F32, tag="rsd")
                nc.vector.reciprocal(rsd, sd)
                y1 = ow.tile([P, D], F32, tag="y1")
                nc.vector.tensor_scalar(
                    y1, y0, scalar1=mv[:, 0:1], scalar2=rsd,
                    op0=mybir.AluOpType.subtract, op1=mybir.AluOpType.mult)
                nc.vector.tensor_mul(y1, y1, gam_b)
                nc.gpsimd.tensor_add(y1, y1, bet_b)
                nc.sync.dma_start(y_d[bass.ds(m * P, P), :], y1)

    nc.compile()
    return nc


def _prepare(query, key, value, causal_mask, wq, bq, wk, bk, wv, bv,
             wo, bo, gamma, beta):
    query = np.asarray(query, np.float32)
    key = np.asarray(key, np.float32)
    value = np.asarray(value, np.float32)
    mask = np.asarray(causal_mask)
    wq, bq = np.asarray(wq, np.float32), np.asarray(bq, np.float32)
    wk, bk = np.asarray(wk, np.float32), np.asarray(bk, np.float32)
    wv, bv = np.asarray(wv, np.float32), np.asarray(bv, np.float32)
    wo, bo = np.asarray(wo, np.float32), np.asarray(bo, np.float32)
    gamma, beta = np.asarray(gamma, np.float32), np.asarray(beta, np.float32)

    kinds, mixed = _classify_blocks(mask)
    key_sig = tuple(sorted(kinds.items()))
    if key_sig not in _CACHE:
        _CACHE[key_sig] = _build(kinds, max(len(mixed), 1))
    nc = _CACHE[key_sig]

    mix_list = [mixed[qi, ki] for qi in range(NT) for ki in range(NT)
                if kinds[qi, ki] == "mix"]
    mixm = (np.stack(mix_list) if mix_list
            else np.zeros((1, P, P), np.float32))
    mixmT = np.ascontiguousarray(mixm.transpose(0, 2, 1))
    bo_p = (bv @ wo.T + bo).astype(np.float32)
    shared = {
        "wqT": np.ascontiguousarray(wq.T), "wkT": np.ascontiguousarray(wk.T),
        "wvT": np.ascontiguousarray(wv.T), "woT": np.ascontiguousarray(wo.T),
        "bq8": np.ascontiguousarray(bq.reshape(NT, P).T),
        "bk8": np.ascontiguousarray(bk.reshape(NT, P).T),
        "gam": np.ascontiguousarray(gamma.reshape(1, D)),
        "bet": np.ascontiguousarray(beta.reshape(1, D)),
        "mixm": np.ascontiguousarray(mixm),
        "mixmT": mixmT,
    }
    in_maps = [{
        "xT": np.ascontiguousarray(query[b].T),
        "kT": np.ascontiguousarray(key[b].T),
        "vT": np.ascontiguousarray(value[b].T),
        "xq": np.ascontiguousarray(query[b] + bo_p),
        **shared,
    } for b in range(B)]
    return nc, in_maps


def kernel(**inputs):
    nc, in_maps = _prepare(**inputs)
    res = run_bass_kernel_spmd(nc, in_maps, list(range(B)))
    y = np.stack([res.results[b]["y"] for b in range(B)])
    attn = np.stack([res.results[b]["attn"] for b in range(B)])
    return y.astype(np.float32), attn.astype(np.float32)


def run_traced(inputs):
    """Like kernel() but with NTFF tracing; returns BassKernelResults."""
    nc, in_maps = _prepare(**inputs)
    return run_bass_kernel_spmd(nc, in_maps, list(range(B)), trace=True)


# revision 21
# speedup vs baseline: 2.6559x; 1.0617x over previous
"""Trainium2 Bass kernel for nn_DirectionalAttention (B=8,S=1024,D=1024,H=16).

Sharding: pure data-parallel over batch — 8 batch elements onto 8 NeuronCores,
zero collectives. Each core runs the full attention layer for one batch
element. Host pre-transposes activations/weights so no on-device input
transposes are needed; the out-proj bias is folded as bo' = bv@wo.T + bo
(valid because softmax rows sum to 1) into the residual input.

Per-core dataflow (S=1024, D=1024, H=16, DK=64), f32r matmuls (full PE rate,
~1.5e-4 rel err measured on HW; flip MM_DT to float32 for 1e-7 at 4x cost):
  QT = wq @ x^T [D,S], KT likewise, V = v @ wv^T [S,D]
  per (head, q-tile): scores -> +mask (mixed blocks) -> ACT Exp(scale=1/8,
    accum_out=rowsum) -> reciprocal -> normalize (DVE 2x) -> DMA out
    (skipped all-masked blocks stay zero: output buffers are pre-zeroed)
    -> PE-transpose 128-blocks -> attnT (f32r)
  AV computed transposed: AVT[d,q] = Vh.T @ attnT — directly the lhsT layout
    the output projection needs; odd heads land on PSUM partitions 64:128
    via tile_position col groups.
  y = LN(AVT.T @ woT + query + bo') * gamma + beta  (bn_stats/bn_aggr)

Self-contained: hardcodes shapes; reads nothing from /root/problem.
"""

import numpy as np

import concourse.bacc as bacc
import concourse.bass as bass
import concourse.mybir as mybir
from concourse.tile import TileContext
from concourse.masks import make_identity
from concourse.bass_utils import run_bass_kernel_spmd

B, S, D, H = 8, 1024, 1024, 16
DK = D // H
P = 128
NT = S // P
LN_EPS = 1e-5
NEG = -1e9

MM_DT = mybir.dt.float16
F32 = mybir.dt.float32
FP16 = mybir.dt.float16
AFT = mybir.ActivationFunctionType

_CACHE: dict = {}


def _classify_blocks(mask: np.ndarray):
    kinds, mixed = {}, {}
    for qi in range(NT):
        for ki in range(NT):
            blk = mask[qi * P:(qi + 1) * P, ki * P:(ki + 1) * P]
            if (blk != 0).all():
                kinds[qi, ki] = "one"
            elif (blk == 0).all():
                kinds[qi, ki] = "zero"
            else:
                kinds[qi, ki] = "mix"
                mixed[qi, ki] = np.where(blk == 0, np.float32(NEG),
                                         np.float32(0.0))
    return kinds, mixed


def _build(kinds, n_mixed):
    nc = bacc.Bacc("TRN2", target_bir_lowering=False, debug=False,
                   num_devices=B)
    dp = nc.declare_dram_parameter
    xT = dp("xT", [D, S], F32, isOutput=False)
    kTd = dp("kT", [D, S], F32, isOutput=False)
    vT = dp("vT", [D, S], F32, isOutput=False)
    xq = dp("xq", [S, D], F32, isOutput=False)
    wqT = dp("wqT", [D, D], F32, isOutput=False)
    wkT = dp("wkT", [D, D], F32, isOutput=False)
    wvT = dp("wvT", [D, D], F32, isOutput=False)
    woT = dp("woT", [D, D], F32, isOutput=False)
    bq8 = dp("bq8", [P, NT], F32, isOutput=False)
    bk8 = dp("bk8", [P, NT], F32, isOutput=False)
    gam = dp("gam", [1, D], F32, isOutput=False)
    bet = dp("bet", [1, D], F32, isOutput=False)
    mixm = dp("mixm", [max(n_mixed, 1), P, P], F32, isOutput=False)
    mixmT = dp("mixmT", [max(n_mixed, 1), P, P], F32, isOutput=False)
    y_d = dp("y", [S, D], F32, isOutput=True)
    attn_d = dp("attn", [H, S, S], F32, isOutput=True)

    kblocks = {qi: [ki for ki in range(NT) if kinds[qi, ki] != "zero"]
               for qi in range(NT)}
    for qi in range(NT):
        assert kblocks[qi], "fully-masked row tile: softmax undefined"
    mix_idx = {}
    for qi in range(NT):
        for ki in range(NT):
            if kinds[qi, ki] == "mix":
                mix_idx[qi, ki] = len(mix_idx)

    with TileContext(nc) as tc:
      with tc.tile_pool(name="consts", bufs=1) as cp, \
           tc.tile_pool(name="stat", bufs=6) as st:
        ident = cp.tile([P, P], F32)
        make_identity(nc, ident)
        gam_b = cp.tile([P, D], F32)
        bet_b = cp.tile([P, D], F32)
        with tc.tile_pool(name="rowstage", bufs=1) as rsp:
            grow = rsp.tile([1, D], F32, tag="grow")
            brow = rsp.tile([1, D], F32, tag="brow")
            nc.sync.dma_start(grow, gam[:, :])
            nc.sync.dma_start(brow, bet[:, :])
            nc.gpsimd.partition_broadcast(gam_b, grow)
            nc.gpsimd.partition_broadcast(bet_b, brow)
        bq_sb = cp.tile([P, NT], F32)
        bk_sb = cp.tile([P, NT], F32)
        nc.sync.dma_start(bq_sb, bq8[:, :])
        nc.sync.dma_start(bk_sb, bk8[:, :])
        eps_sb = cp.tile([P, 1], F32)
        nc.vector.memset(eps_sb, LN_EPS)
        masks_sb = cp.tile([P, max(n_mixed, 1), P], F32)
        nc.sync.dma_start(masks_sb, mixm[:, :, :].rearrange("n p m -> p n m"))
        masksT_sb = cp.tile([P, max(n_mixed, 1), P], F32)
        nc.sync.dma_start(masksT_sb, mixmT[:, :, :].rearrange("n p m -> p n m"))

        with tc.tile_pool(name="avt", bufs=1) as ap:
          AVT = ap.tile([P, NT, S], FP16)

          with tc.tile_pool(name="qkv", bufs=1) as qp:
            QT = qp.tile([P, NT, S], MM_DT)
            KT = qp.tile([P, NT, S], MM_DT)
            V = qp.tile([P, NT, D], FP16)

            # ============ projections (k-outer, 8 live psum banks) ========
            with tc.tile_pool(name="wres", bufs=1) as wr, \
                 tc.tile_pool(name="praw", bufs=3) as raw, \
                 tc.tile_pool(name="pcast", bufs=3) as cst, \
                 tc.tile_pool(name="ppsum", bufs=8, space="PSUM") as pps:

                def project(w_dram, x_dram, out_sb, bias_sb, w_is_lhsT):
                    w_r = wr.tile([P, NT, D], MM_DT, tag="wres")
                    for k in range(NT):
                        wt = raw.tile([P, D], F32, tag="rawW")
                        nc.sync.dma_start(wt, w_dram[bass.ds(k * P, P), :])
                        nc.vector.tensor_copy(w_r[:, k], wt)
                    for mg in range(2):
                        pss = [pps.tile([P, 512], F32, tag="pj",
                                        name=f"pj{i}")
                               for i in range(8)]
                        for k in range(NT):
                            xt = raw.tile([P, S], F32, tag="rawX")
                            nc.sync.dma_start(
                                xt, x_dram[bass.ds(k * P, P), :])
                            xc = cst.tile([P, S], MM_DT, tag="castX")
                            nc.vector.tensor_copy(xc, xt)
                            for mi in range(4):
                                m = mg * 4 + mi
                                for n in range(2):
                                    if w_is_lhsT:
                                        lhsT = w_r[:, k, m * P:(m + 1) * P]
                                        rhs = xc[:, n * 512:(n + 1) * 512]
                                    else:
                                        lhsT = xc[:, m * P:(m + 1) * P]
                                        rhs = w_r[:, k, n * 512:(n + 1) * 512]
                                    nc.tensor.matmul(
                                        pss[mi * 2 + n], lhsT, rhs,
                                        start=(k == 0), stop=(k == NT - 1))
                        for mi in range(4):
                            m = mg * 4 + mi
                            for n in range(2):
                                dst = out_sb[:, m, n * 512:(n + 1) * 512]
                                ps = pss[mi * 2 + n]
                                if bias_sb is not None:
                                    nc.vector.tensor_scalar_add(
                                        dst, ps, bias_sb[:, m:m + 1])
                                else:
                                    nc.vector.tensor_copy(dst, ps)

                project(wqT, xT, QT, bq_sb, True)
                project(wkT, kTd, KT, bk_sb, True)
                project(wvT, vT, V, None, False)

            # ===================== attention ==============================
            with tc.tile_pool(name="atile", bufs=1) as tp, \
                 tc.tile_pool(name="awork", bufs=3) as wkp, \
                 tc.tile_pool(name="psc", bufs=2, space="PSUM") as psc, \
                 tc.tile_pool(name="pstr", bufs=1, space="PSUM") as pstp, \
                 tc.tile_pool(name="pav", bufs=2, space="PSUM") as pavp:
                attnT = tp.tile([P, NT, S], FP16)
                rinv_hq = tp.tile([P, H * NT], F32)
                rinvT = tp.tile([P, H * NT], F32)
                zero_f = tp.tile([P, S], F32, tag="zf")
                nc.vector.memset(zero_f, 0.0)
                for ki in range(NT):
                    if any(kinds[qi, ki] == "zero" for qi in range(NT)):
                        nc.vector.tensor_copy(attnT[:, ki], zero_f)

                # Merged per-head loop with depth-1 skew: PE stays
                # dense (scores(h) -> scoresT(h) -> AV(h-1)) so ACT exp
                # latency hides and HAM stays warm. attnT double-buffered
                # so expT(h) doesn't clobber what AV(h-1) reads.
                attnT_bufs = [attnT, tp.tile([P, NT, S], FP16,
                                             name="attnT2")]
                for buf in attnT_bufs:
                    for ki in range(NT):
                        if any(kinds[qi, ki] == "zero"
                               for qi in range(NT)):
                            nc.vector.tensor_copy(buf[:, ki], zero_f)

                def emit_pass1(h):
                    ht, hp = divmod(h * DK, P)
                    for qi in range(NT):
                        kbs = kblocks[qi]
                        wmax = (max(kbs) + 1) * P
                        ps = psc.tile([P, S], F32, tag="psc", name="ps")
                        for g in range(2):
                            grp = [ki for ki in kbs
                                   if g * 4 <= ki < g * 4 + 4]
                            while grp:
                                a = grp[0]
                                b2 = a
                                while b2 + 1 in grp:
                                    b2 += 1
                                grp = [x for x in grp if x > b2]
                                nc.tensor.matmul(
                                    ps[:, a * P:(b2 + 1) * P],
                                    QT[hp:hp + DK, ht,
                                       qi * P:(qi + 1) * P],
                                    KT[hp:hp + DK, ht, a * P:(b2 + 1) * P],
                                    start=True, stop=True)
                        for ki in range(wmax // P):
                            if ki not in kbs:
                                nc.vector.memset(
                                    ps[:, ki * P:(ki + 1) * P], NEG)
                            elif kinds[qi, ki] == "mix":
                                nc.vector.tensor_add(
                                    ps[:, ki * P:(ki + 1) * P],
                                    ps[:, ki * P:(ki + 1) * P],
                                    masks_sb[:, mix_idx[qi, ki]])
                        ex = wkp.tile([P, S], F32, tag="exp", name="ex")
                        rs = st.tile([P, 1], F32, tag="rs", name="rs")
                        at = wkp.tile([P, S], F32, tag="attn", name="at")
                        nc.scalar.activation(
                            ex[:, :wmax], ps[:, :wmax], AFT.Exp,
                            scale=0.125, accum_out=rs)
                        hq = h * NT + qi
                        nc.vector.reciprocal(rinv_hq[:, hq:hq + 1], rs)
                        nc.vector.tensor_scalar_mul(
                            at[:, :wmax], ex[:, :wmax],
                            rinv_hq[:, hq:hq + 1])
                        nc.sync.dma_start(
                            attn_d[h, bass.ds(qi * P, P), 0:wmax],
                            at[:, :wmax])

                def emit_rbc(h):
                    pstr = pstp.tile([P, P], F32, tag="pstr", name="pstr")
                    nc.tensor.transpose(pstr, rinv_hq, ident)
                    nc.vector.tensor_copy(rinvT, pstr)
                    rbcs = []
                    for c in range(2):
                        hq0 = h * NT + 4 * c
                        rstage = wkp.tile([1, 512], F32, tag="rstg",
                                          name="rstage")
                        nc.gpsimd.dma_start(rstage,
                                            rinvT[hq0:hq0 + 4, :])
                        rbc = wkp.tile([DK, 512], F32, tag="rbc",
                                       name="rbc", bufs=4)
                        nc.gpsimd.partition_broadcast(rbc, rstage)
                        rbcs.append(rbc)
                    return rbcs

                def emit_scoresT(h, buf):
                    ht, hp = divmod(h * DK, P)
                    for c in range(2):
                        c0 = c * 512
                        for ki in range(NT):
                            qs = [qi for qi in range(4 * c, 4 * c + 4)
                                  if kinds[qi, ki] != "zero"]
                            if not qs:
                                continue
                            if qs != list(range(qs[0], qs[-1] + 1)):
                                raise NotImplementedError(
                                    "non-contiguous valid q-blocks")
                            qlo = qs[0] * P - c0
                            qhi = (qs[-1] + 1) * P - c0
                            psT = pavp.tile([P, 512], F32, tag="psT",
                                            name="psT")
                            nc.tensor.matmul(
                                psT[:, qlo:qhi],
                                KT[hp:hp + DK, ht, ki * P:(ki + 1) * P],
                                QT[hp:hp + DK, ht, c0 + qlo:c0 + qhi],
                                start=True, stop=True)
                            for qi in qs:
                                a = qi * P - c0
                                if kinds[qi, ki] == "mix":
                                    nc.vector.tensor_add(
                                        psT[:, a:a + P], psT[:, a:a + P],
                                        masksT_sb[:, mix_idx[qi, ki]])
                            nc.scalar.activation(
                                buf[:, ki, c0 + qlo:c0 + qhi],
                                psT[:, qlo:qhi], AFT.Exp, scale=0.125)

                def emit_av(h, buf, rbcs):
                    ht, hp = divmod(h * DK, P)
                    for c in range(2):
                        kis = sorted({k for qi in range(4 * c, 4 * c + 4)
                                      for k in kblocks[qi]})
                        pa = pavp.tile([DK, 512], F32, tag="pav",
                                       name="pa", bufs=1)
                        for j, ki in enumerate(kis):
                            nc.tensor.matmul(
                                pa, V[:, ki, h * DK:(h + 1) * DK],
                                buf[:, ki, c * 512:(c + 1) * 512],
                                start=(j == 0), stop=(j == len(kis) - 1))
                        if hp == 0:
                            nc.vector.tensor_mul(
                                AVT[0:DK, ht, c * 512:(c + 1) * 512],
                                pa, rbcs[c])
                        else:
                            bounce = wkp.tile([DK, 512], FP16, tag="bnc",
                                              name="bounce")
                            nc.vector.tensor_mul(bounce, pa, rbcs[c])
                            nc.sync.dma_start(
                                AVT[hp:hp + DK, ht,
                                    c * 512:(c + 1) * 512], bounce)

                prev = None
                for h in range(H):
                    buf = attnT_bufs[h % 2]
                    emit_pass1(h)
                    rbcs = emit_rbc(h)
                    emit_scoresT(h, buf)
                    if prev is not None:
                        emit_av(*prev)
                    prev = (h, buf, rbcs)
                assert prev is not None
                emit_av(*prev)

          # ============== output projection + residual + LN ===============
          with tc.tile_pool(name="ores", bufs=1) as owr, \
               tc.tile_pool(name="oraw", bufs=3) as oraw, \
               tc.tile_pool(name="owork", bufs=2) as ow, \
               tc.tile_pool(name="opsum", bufs=4, space="PSUM") as ops:
            wo_r = owr.tile([P, NT, D], FP16)
            for k in range(NT):
                wt = oraw.tile([P, D], F32, tag="rawW")
                nc.sync.dma_start(wt, woT[bass.ds(k * P, P), :])
                nc.vector.tensor_copy(wo_r[:, k], wt)
            for m in range(NT):
                xr = oraw.tile([P, D], F32, tag="rawX")
                nc.sync.dma_start(xr, xq[bass.ds(m * P, P), :])
                y0 = ow.tile([P, D], F32, tag="y0")
                for n in range(2):
                    ps = ops.tile([P, 512], F32, tag="oj")
                    for k in range(NT):
                        nc.tensor.matmul(
                            ps, AVT[:, k, m * P:(m + 1) * P],
                            wo_r[:, k, n * 512:(n + 1) * 512],
                            start=(k == 0), stop=(k == NT - 1))
                    nc.vector.tensor_add(
                        y0[:, n * 512:(n + 1) * 512], ps,
                        xr[:, n * 512:(n + 1) * 512])
                stats = st.tile([P, 2, 6], F32, tag="bns")
                mv = st.tile([P, 2], F32, tag="mv")
                for sg in range(2):
                    nc.vector.bn_stats(stats[:, sg],
                                       y0[:, sg * 512:(sg + 1) * 512])
                nc.vector.bn_aggr(mv, stats)
                sd = st.tile([P, 1], F32, tag="sd")
                nc.scalar.activation(sd, mv[:, 1:2], AFT.Sqrt, bias=eps_sb)
                rsd = st.tile([P, 1], F32, tag="rsd")
                nc.vector.reciprocal(rsd, sd)
                y1 = ow.tile([P, D], F32, tag="y1")
                nc.vector.tensor_scalar(
                    y1, y0, scalar1=mv[:, 0:1], scalar2=rsd,
                    op0=mybir.AluOpType.subtract, op1=mybir.AluOpType.mult)
                nc.vector.tensor_mul(y1, y1, gam_b)
                nc.gpsimd.tensor_add(y1, y1, bet_b)
                nc.sync.dma_start(y_d[bass.ds(m * P, P), :], y1)

    nc.compile()
    return nc


def _prepare(query, key, value, causal_mask, wq, bq, wk, bk, wv, bv,
             wo, bo, gamma, beta):
    query = np.asarray(query, np.float32)
    key = np.asarray(key, np.float32)
    value = np.asarray(value, np.float32)
    mask = np.asarray(causal_mask)
    wq, bq = np.asarray(wq, np.float32), np.asarray(bq, np.float32)
    wk, bk = np.asarray(wk, np.float32), np.asarray(bk, np.float32)
    wv, bv = np.asarray(wv, np.float32), np.asarray(bv, np.float32)
    wo, bo = np.asarray(wo, np.float32), np.asarray(bo, np.float32)
    gamma, beta = np.asarray(gamma, np.float32), np.asarray(beta, np.float32)

    kinds, mixed = _classify_blocks(mask)
    key_sig = tuple(sorted(kinds.items()))
    if key_sig not in _CACHE:
        _CACHE[key_sig] = _build(kinds, max(len(mixed), 1))
    nc = _CACHE[key_sig]

    mix_list = [mixed[qi, ki] for qi in range(NT) for ki in range(NT)
                if kinds[qi, ki] == "mix"]
    mixm = (np.stack(mix_list) if mix_list
            else np.zeros((1, P, P), np.float32))
    mixmT = np.ascontiguousarray(mixm.transpose(0, 2, 1))
    bo_p = (bv @ wo.T + bo).astype(np.float32)
    shared = {
        "wqT": np.ascontiguousarray(wq.T), "wkT": np.ascontiguousarray(wk.T),
        "wvT": np.ascontiguousarray(wv.T), "woT": np.ascontiguousarray(wo.T),
        "bq8": np.ascontiguousarray(bq.reshape(NT, P).T),
        "bk8": np.ascontiguousarray(bk.reshape(NT, P).T),
        "gam": np.ascontiguousarray(gamma.reshape(1, D)),
        "bet": np.ascontiguousarray(beta.reshape(1, D)),
        "mixm": np.ascontiguousarray(mixm),
        "mixmT": mixmT,
    }
    in_maps = [{
        "xT": np.ascontiguousarray(query[b].T),
        "kT": np.ascontiguousarray(key[b].T),
        "vT": np.ascontiguousarray(value[b].T),
        "xq": np.ascontiguousarray(query[b] + bo_p),
        **shared,
    } for b in range(B)]
    return nc, in_maps


def kernel(**inputs):
    nc, in_maps = _prepare(**inputs)
    res = run_bass_kernel_spmd(nc, in_maps, list(range(B)))
    y = np.stack([res.results[b]["y"] for b in range(B)])
    attn = np.stack([res.results[b]["attn"] for b in range(B)])
    return y.astype(np.float32), attn.astype(np.float32)


def run_traced(inputs):
    """Like kernel() but with NTFF tracing; returns BassKernelResults."""
    nc, in_maps = _prepare(**inputs)
    return run_bass_kernel_spmd(nc, in_maps, list(range(B)), trace=True)
